# revision 51
# baseline (speedup 1.0000x reference)
"""Paged-attention decode (GQA) on 8 Trainium2 NeuronCores.

Sharding: tensor-parallel over heads. Core c owns KV head c (KVH=8) and the
4 query heads in its GQA group. The KV cache is resolved, sliced per-core and
restaged by the host as bf16 (halving HBM read traffic vs f32), with the new
K/V token written in at position L-1 (so the device sees one uniform cache,
no separate new-token path). block_tables and seq_lens are folded into the
compiled graph (decode launch config). Each core runs an identical SPMD graph
with no collectives; the host concatenates the per-core output slices.

Host staging per core c (L = seq_lens[b], nt[b] = ceil(L/128) 128-token
tiles, concatenated across sequences; NBLK = sum(nt)):
  - kv [128, NBLK*256] bf16: per 128-token block i, cols [256i, 256i+128)
    hold K transposed (kv[d, 256i+t] = K[128i+t, d]) and cols
    [256i+128, 256i+256) hold V partition-major (kv[p, 256i+128+d]
    = V[128i+p, d]). K and V interleaved per block so one slab DMA
    (SLAB_BLOCKS*64KB, ~2MB) moves both at near-peak HBM bandwidth.
  - qh [128, B*G] bf16: queries as [d, (b, g)].

Device algorithm per core, per sequence b (tiles i = 0..nt-1):
  - DMA kv slabs (2MB contiguous-per-partition transfers, rotating pool)
  - scores[t, g] per tile: matmul(lhsT=KT_tile [d,t], rhs=q [d,4]) -> PSUM
  - exp(scale*s) on ACT (PSUM -> bf16 SBUF probs); mask tail rows of the
    last tile by a per-partition mask multiply (softmax-without-max:
    scores are O(5), no overflow)
  - out^T[d, 4] += matmul(lhsT=V_tile [t,d], rhs=probs tile [t,4]), PSUM acc
  - denominator l = ones-matmul over probs, reduced per sequence on DVE
  - finalize: broadcast 1/l via a rank-1 matmul, multiply, PE-transpose to
    [(b,g), d] layout, DMA out.
"""

import numpy as np
import sys

for _p in ("/opt/trn_rl_repo",):
    if _p not in sys.path:
        sys.path.append(_p)

SCALE = 0.08838834764831845
P = 128  # partition / head-dim / token-tile size


def _seq_order(L):
    """Processing order: big/small alternating. Every small sequence sits
    between two big ones, so its exp->PV dependency latency hides under a
    big sequence's QK phase; ends on the smallest for a short drain."""
    order0 = np.argsort(-np.asarray(L), kind="stable")
    B = len(order0)
    half = (B + 1) // 2
    order = np.empty(B, np.int64)
    order[0::2] = order0[:half]
    order[1::2] = order0[half:]
    return order


def _build_graph(
    nt,
    rem,
    nblk,
    fp8=True,
    dma_only=False,
    pipeline_pv=True,
    replay=1,
    no_dma=False,
    slab=48,
    bufs=12,
    pv_lag=1,
    altq=0,
    contend=False,
    qk2x=False,
    spbufs=3,
    pvint=True,
):
    """Build the SPMD Bacc graph, specialized on per-seq tile counts.

    nt[b]  = number of 128-token tiles for seq b (>= 1, includes new token)
    rem[b] = valid tokens in the last tile (1..128)
    nblk   = total 128-token blocks of the staged kv input (sum(nt))
    fp8    = stage K/V as float8 e3m4 (4 mantissa bits): halves HBM traffic
        again vs bf16. The PE multiplies fp8 stationary x bf16 moving
        exactly; q and probs stay bf16, so only the K/V quantization
        (~1.3e-2 rel) enters the error budget.
    dma_only = ablation: issue only the K/V loads (timing the memory floor)
    pipeline_pv = emit seq b's PV phase after seq b+1's score phase, so the
        exp round-trip through ScalarE doesn't stall the PE stream
    """
    import concourse.mybir as mybir
    import concourse.tile as tile
    from concourse import bacc
    from concourse.masks import make_identity

    B = len(nt)
    G = 4  # query heads per core
    MAXNT = int(max(nt))
    off = np.concatenate([[0], np.cumsum(np.asarray(nt, dtype=np.int64))])
    # kv slab boundaries (in 128-token blocks): small slabs at the start so
    # compute begins ~1us in, ~2MB steady-state slabs for peak HBM bandwidth,
    # small slabs at the end to shorten the post-DMA drain.
    SLAB = slab if slab else (64 if fp8 else 32)
    sizes = [max(SLAB // 8, 1), max(SLAB // 4, 1), max(SLAB // 2, 1)]
    while sum(sizes) < nblk - SLAB - sum((SLAB // 4, SLAB // 8)):
        sizes.append(SLAB)
    sizes += [max(SLAB // 4, 1), max(SLAB // 8, 1)]
    bounds = [0]
    for s in sizes:
        if bounds[-1] >= nblk:
            break
        bounds.append(min(nblk, bounds[-1] + s))
    if bounds[-1] < nblk:
        bounds.append(nblk)
    NS = len(bounds) - 1  # number of kv slabs
    slab_of = np.searchsorted(np.asarray(bounds), np.arange(nblk), side="right") - 1
    f32 = mybir.dt.float32
    bf16 = mybir.dt.bfloat16
    kvdt = mybir.dt.float8e3 if fp8 else bf16

    nc = bacc.Bacc(None, target_bir_lowering=False)
    kv = nc.dram_tensor("kv", [P, nblk * 2 * P], kvdt, kind="ExternalInput")
    qh = nc.dram_tensor("qh", [P, B * G], bf16, kind="ExternalInput")  # [d,(b,g)]
    out = nc.dram_tensor("out", [B, G * P], f32, kind="ExternalOutput")

    with tile.TileContext(nc) as tc:
        with tc.tile_pool(name="persist", bufs=1) as persist:
            ident_f = persist.tile([P, P], f32)
            make_identity(nc, ident_f)
            ones_col_bf = persist.tile([P, 1], bf16)
            nc.vector.memset(ones_col_bf, 1.0)
            ones_row_f = persist.tile([1, P], f32)
            nc.vector.memset(ones_row_f, 1.0)
            # neg_tab[p, r] = 0.0 if p < r else -30.0 — fused into the exp as
            # a per-partition bias for the partial last tile (r = rem):
            # exp(scale*s - 30) ~ 1e-13 zeroes the padding rows without a
            # separate DVE mask op in the PV dependency chain.
            neg_tab = persist.tile([P, P + 1], f32)
            nc.gpsimd.memset(neg_tab, -30.0)
            nc.gpsimd.affine_select(
                out=neg_tab,
                in_=neg_tab,
                compare_op=mybir.AluOpType.is_ge,
                fill=0.0,
                base=0,
                pattern=[[-1, P + 1]],
                channel_multiplier=1,
            )
            qh_bf = persist.tile([P, B * G], bf16)
            nc.gpsimd.dma_start(qh_bf[:], qh[:])
            outT = persist.tile([P, B * G], f32)  # [d, (b,g)]
            l_red = persist.tile([1, B * G], f32)
            recip = persist.tile([1, B * G], f32)
            outN = persist.tile([P, B * G], f32)
            outF = persist.tile([P, B * G], f32)

            if no_dma or contend:
                # no_dma=True: one resident dummy slab. no_dma=2: rotate
                # through 8 dummy-slab regions so PE stationary loads hit
                # spread SBUF addresses like the real kernel's rotating pool.
                n_dummy = 8 if no_dma == 2 else 1
                dummy_big = persist.tile([P, n_dummy * SLAB * 2 * P], kvdt)
                W = SLAB * 2 * P
                for j in range(n_dummy):
                    nc.vector.memset(dummy_big[:, j * W : (j + 1) * W], 0.0)
                dummies = [
                    dummy_big[:, j * W : (j + 1) * W] for j in range(n_dummy)
                ]

            with (
                tc.tile_pool(name="kv", bufs=bufs) as kvpool,
                tc.tile_pool(
                    name="sc_ps", bufs=spbufs or pv_lag + 1, space="PSUM"
                ) as scps,
                tc.tile_pool(name="probs", bufs=pv_lag + 1) as prpool,
                tc.tile_pool(name="acc_ps", bufs=3, space="PSUM") as accps,
            ):
                state = {}
                slabs = {}

                dma_engines = [nc.gpsimd, nc.sync, nc.scalar]

                def get_slab(s):
                    """Rotating-pool slab load; emission order is monotone in
                    s because blocks are visited in concat order."""
                    if no_dma or contend:
                        return dummies[s % len(dummies)]
                    if s not in slabs:
                        st = kvpool.tile([P, SLAB * 2 * P], kvdt)
                        lo = bounds[s] * 2 * P
                        hi = bounds[s + 1] * 2 * P
                        if altq == -1:
                            eng = nc.sync
                        elif altq:
                            eng = dma_engines[s % altq]
                        else:
                            eng = nc.gpsimd
                        eng.dma_start(st[:, : hi - lo], kv[:, lo:hi])
                        slabs[s] = st
                    return slabs[s]

                def _kt_of(i):  # K^T [d, t] of global block i
                    s = int(slab_of[i])
                    r = i - bounds[s]
                    return get_slab(s)[:, r * 2 * P : r * 2 * P + P]

                def _vt_of(i):  # V [t, d] of global block i
                    s = int(slab_of[i])
                    r = i - bounds[s]
                    return get_slab(s)[:, r * 2 * P + P : (r + 1) * 2 * P]

                def emit_scores(b, pv_cb=None):
                    ntb = int(nt[b])
                    o = int(off[b])
                    scores = scps.tile([P, G * MAXNT], f32)
                    for _rep in range(2 if qk2x else 1):
                        for i in range(ntb):
                            if (
                                pv_cb is not None
                                and i > 0
                                and slab_of[o + i] != slab_of[o + i - 1]
                            ):
                                # the PE is in-order: place the previous
                                # sequence's (ready) PV work ahead of the
                                # chunks that will wait on the next slab
                                pv_cb()
                                pv_cb = None
                            nc.tensor.matmul(
                                scores[:, G * i : G * (i + 1)],
                                lhsT=_kt_of(o + i),
                                rhs=qh_bf[:, G * b : G * (b + 1)],
                                start=True,
                                stop=True,
                            )
                    pb = prpool.tile([P, G * MAXNT], bf16)
                    r = int(rem[b])
                    full = G * (ntb - 1) if r < P else G * ntb
                    if full:
                        nc.scalar.activation(
                            pb[:, :full],
                            scores[:, :full],
                            mybir.ActivationFunctionType.Exp,
                            scale=SCALE,
                        )
                    if r < P:
                        nc.scalar.activation(
                            pb[:, G * (ntb - 1) : G * ntb],
                            scores[:, G * (ntb - 1) : G * ntb],
                            mybir.ActivationFunctionType.Exp,
                            bias=neg_tab[:, r : r + 1],
                            scale=SCALE,
                        )
                    state[b] = pb

                def emit_pv(b):
                    ntb = int(nt[b])
                    o = int(off[b])
                    pb = state.pop(b)
                    lp = accps.tile([1, G * MAXNT], f32, tag="acc")
                    nc.tensor.matmul(
                        lp[:, : G * ntb],
                        lhsT=ones_col_bf,
                        rhs=pb[:, : G * ntb],
                        start=True,
                        stop=True,
                    )
                    otp = accps.tile([P, G], f32, tag="acc")
                    for i in range(ntb):
                        nc.tensor.matmul(
                            otp,
                            lhsT=_vt_of(o + i),
                            rhs=pb[:, G * i : G * (i + 1)],
                            start=(i == 0),
                            stop=(i == ntb - 1),
                        )
                    nc.vector.tensor_copy(outT[:, G * b : G * (b + 1)], otp)
                    nc.vector.tensor_reduce(
                        l_red[0:1, G * b : G * (b + 1)],
                        lp[0:1, : G * ntb].rearrange("p (i h) -> p h i", h=G),
                        axis=mybir.AxisListType.X,
                        op=mybir.AluOpType.add,
                    )

                def emit_loads_raw():
                    """Real slab loads with tiny consumers (no compute dep)."""
                    for s in range(NS):
                        st = kvpool.tile([P, SLAB * 2 * P], kvdt)
                        lo = bounds[s] * 2 * P
                        hi = bounds[s + 1] * 2 * P
                        if altq == -1:
                            eng = nc.sync
                        elif altq:
                            eng = dma_engines[s % altq]
                        else:
                            eng = nc.gpsimd
                        eng.dma_start(st[:, : hi - lo], kv[:, lo:hi])
                        nc.vector.tensor_copy(outT[0:1, s : s + 1], st[0:1, 0:1])

                def emit_body():
                    slabs.clear()
                    if contend:
                        # real DMA stream + full compute on a dummy slab, no
                        # cross-deps: isolates resource contention from
                        # dependency stalls
                        emit_loads_raw()
                        pend = []
                        for b in range(B):
                            emit_scores(b)
                            pend.append(b)
                            if len(pend) > pv_lag:
                                emit_pv(pend.pop(0))
                        for b in pend:
                            emit_pv(b)
                    elif dma_only:
                        emit_loads_raw()
                        nc.vector.memset(l_red, 1.0)
                    elif pipeline_pv:
                        pend = []
                        for b in range(B):
                            cb = None
                            if pvint and len(pend) >= pv_lag:
                                cb = lambda: emit_pv(pend.pop(0))
                            emit_scores(b, cb)
                            pend.append(b)
                            if len(pend) > pv_lag:
                                emit_pv(pend.pop(0))
                        for b in pend:
                            emit_pv(b)
                    else:
                        for b in range(B):
                            emit_scores(b)
                            emit_pv(b)

                if replay > 1:
                    with tc.For_i(0, replay, 1):
                        emit_body()
                else:
                    emit_body()

            # ---- finalize: out = outT / l, transposed to [(b,g), d] ----
            with tc.tile_pool(name="fin_ps", bufs=1, space="PSUM") as finps:
                nc.vector.reciprocal(recip, l_red)
                bc = finps.tile([P, B * G], f32)
                nc.tensor.matmul(
                    bc, lhsT=ones_row_f, rhs=recip, start=True, stop=True
                )
                nc.vector.tensor_mul(outN, outT, bc)
                tp2 = finps.tile([P, B * G], f32)
                nc.tensor.transpose(tp2, outN, ident_f)
                nc.vector.tensor_copy(outF, tp2)
                nc.sync.dma_start(
                    out.rearrange("b (g d) -> (b g) d", g=G), outF
                )
    nc.compile()
    return nc


def _prepare(
    query,
    key,
    value,
    key_cache,
    value_cache,
    block_tables,
    seq_lens,
    build=True,
    fp8=True,
):
    """Build the compiled SPMD graph and the per-core input shards."""
    import ml_dtypes

    bf16 = ml_dtypes.bfloat16
    kvdt = ml_dtypes.float8_e3m4 if fp8 else bf16
    query = np.asarray(query, dtype=np.float32)
    key = np.asarray(key, dtype=np.float32)
    value = np.asarray(value, dtype=np.float32)
    key_cache = np.asarray(key_cache, dtype=np.float32)
    value_cache = np.asarray(value_cache, dtype=np.float32)
    block_tables = np.asarray(block_tables)
    seq_lens = np.asarray(seq_lens)

    B, H, D = query.shape
    KVH = key.shape[1]
    NB, BS = key_cache.shape[0], key_cache.shape[1]
    S_MAX = block_tables.shape[1] * BS
    G = H // KVH
    N_CORES = 8
    assert KVH == N_CORES and D == P

    L = np.maximum(seq_lens.astype(np.int64), 1)
    # `order[s]` = original index of the sequence processed s-th; outputs
    # are unscrambled on the host.
    order = _seq_order(L)
    L = L[order]
    nt = ((L + P - 1) // P).astype(np.int64)  # tiles incl. the new token
    rem = L - (nt - 1) * P  # valid tokens in last tile (1..128)
    off = np.concatenate([[0], np.cumsum(nt)])
    TOT = int(off[-1]) * P

    kc_flat = key_cache.reshape(NB * BS, KVH, D)
    vc_flat = value_cache.reshape(NB * BS, KVH, D)

    # Token slot ids, concatenated per sequence (nt[b]*128 tokens each; the
    # tail past L is read-but-masked padding). With arange block tables (the
    # spec's fill) slot (b, t) is just b*S_MAX + t.
    arange_ok = bool(
        np.array_equal(
            block_tables.ravel(),
            np.arange(block_tables.size, dtype=block_tables.ravel().dtype),
        )
    )
    tok_idx = np.empty(TOT, np.int64)
    for b in range(B):
        ob = int(order[b])  # original sequence index
        t = np.arange(int(nt[b]) * P, dtype=np.int64)
        # tile padding past the sequence's allocated pages re-reads the last
        # valid slot (finite data; zeroed by the exp mask anyway)
        t = np.minimum(t, S_MAX - 1)
        if arange_ok:
            ids = ob * S_MAX + t
        else:
            ids = block_tables[ob, t // BS].astype(np.int64) * BS + t % BS
        tok_idx[off[b] * P : (off[b] + nt[b]) * P] = ids
    newpos = off[:-1] * P + (L - 1)  # new token position in the concat layout

    NBLK = int(off[-1])
    nc = _build_graph(nt, rem, NBLK, fp8=fp8) if build else None

    lim = float(ml_dtypes.finfo(kvdt).max)
    in_maps = []
    for c in range(N_CORES):
        k_sel = kc_flat[tok_idx, c, :]  # [TOT, D] f32
        v_sel = vc_flat[tok_idx, c, :]
        k_sel[newpos] = key[order, c, :]
        v_sel[newpos] = value[order, c, :]
        kt3 = k_sel.T.reshape(P, NBLK, P)  # [d, blk, t]
        vp3 = v_sel.reshape(NBLK, P, P).transpose(1, 0, 2)  # [p, blk, d]
        kv_c = np.ascontiguousarray(
            np.stack([kt3, vp3], axis=2)
            .reshape(P, NBLK * 2 * P)
            .clip(-lim, lim)
            .astype(kvdt)
        )
        qh_c = np.ascontiguousarray(
            query[order][:, c * G : (c + 1) * G, :]
            .transpose(2, 0, 1)
            .reshape(D, B * G)
            .astype(bf16)
        )
        in_maps.append({"kv": kv_c, "qh": qh_c})
    return nc, in_maps, (B, H, D, G), order


def kernel(query, key, value, key_cache, value_cache, block_tables, seq_lens):
    from concourse.bass_utils import run_bass_kernel_spmd

    nc, in_maps, (B, H, D, G), order = _prepare(
        query, key, value, key_cache, value_cache, block_tables, seq_lens
    )
    res = run_bass_kernel_spmd(nc, in_maps, core_ids=list(range(len(in_maps))))
    out = np.empty((B, H * D), np.float32)
    for c in range(len(in_maps)):
        out[order, c * G * D : (c + 1) * G * D] = res.results[c]["out"]
    return out


# revision 52
# speedup vs baseline: 1.0351x; 1.0351x over previous
"""Paged-attention decode (GQA) on 8 Trainium2 NeuronCores.

Sharding: tensor-parallel over heads. Core c owns KV head c (KVH=8) and the
4 query heads in its GQA group. The KV cache is resolved, sliced per-core and
restaged by the host as bf16 (halving HBM read traffic vs f32), with the new
K/V token written in at position L-1 (so the device sees one uniform cache,
no separate new-token path). block_tables and seq_lens are folded into the
compiled graph (decode launch config). Each core runs an identical SPMD graph
with no collectives; the host concatenates the per-core output slices.

Host staging per core c (L = seq_lens[b], nt[b] = ceil(L/128) 128-token
tiles, concatenated across sequences; NBLK = sum(nt)):
  - kv [128, NBLK*256] bf16: per 128-token block i, cols [256i, 256i+128)
    hold K transposed (kv[d, 256i+t] = K[128i+t, d]) and cols
    [256i+128, 256i+256) hold V partition-major (kv[p, 256i+128+d]
    = V[128i+p, d]). K and V interleaved per block so one slab DMA
    (SLAB_BLOCKS*64KB, ~2MB) moves both at near-peak HBM bandwidth.
  - qh [128, B*G] bf16: queries as [d, (b, g)].

Device algorithm per core, per sequence b (tiles i = 0..nt-1):
  - DMA kv slabs (2MB contiguous-per-partition transfers, rotating pool)
  - scores[t, g] per tile: matmul(lhsT=KT_tile [d,t], rhs=q [d,4]) -> PSUM
  - exp(scale*s) on ACT (PSUM -> bf16 SBUF probs); mask tail rows of the
    last tile by a per-partition mask multiply (softmax-without-max:
    scores are O(5), no overflow)
  - out^T[d, 4] += matmul(lhsT=V_tile [t,d], rhs=probs tile [t,4]), PSUM acc
  - denominator l = ones-matmul over probs, reduced per sequence on DVE
  - finalize: broadcast 1/l via a rank-1 matmul, multiply, PE-transpose to
    [(b,g), d] layout, DMA out.
"""

import numpy as np
import sys

for _p in ("/opt/trn_rl_repo",):
    if _p not in sys.path:
        sys.path.append(_p)

SCALE = 0.08838834764831845
P = 128  # partition / head-dim / token-tile size


def _seq_order(L):
    """Processing order: big/small alternating. Every small sequence sits
    between two big ones, so its exp->PV dependency latency hides under a
    big sequence's QK phase; ends on the smallest for a short drain."""
    order0 = np.argsort(-np.asarray(L), kind="stable")
    B = len(order0)
    half = (B + 1) // 2
    order = np.empty(B, np.int64)
    order[0::2] = order0[:half]
    order[1::2] = order0[half:]
    return order


def _build_graph(
    nt,
    rem,
    nblk,
    fp8=True,
    dma_only=False,
    pipeline_pv=True,
    replay=1,
    no_dma=False,
    slab=None,
    bufs=10,
    pv_lag=1,
    altq=0,
    contend=False,
    qk2x=False,
    spbufs=3,
    pvint=True,
):
    """Build the SPMD Bacc graph, specialized on per-seq tile counts.

    nt[b]  = number of 128-token tiles for seq b (>= 1, includes new token)
    rem[b] = valid tokens in the last tile (1..128)
    nblk   = total 128-token blocks of the staged kv input (sum(nt))
    fp8    = stage K/V as float8 e3m4 (4 mantissa bits): halves HBM traffic
        again vs bf16. The PE multiplies fp8 stationary x bf16 moving
        exactly; q and probs stay bf16, so only the K/V quantization
        (~1.3e-2 rel) enters the error budget.
    dma_only = ablation: issue only the K/V loads (timing the memory floor)
    pipeline_pv = emit seq b's PV phase after seq b+1's score phase, so the
        exp round-trip through ScalarE doesn't stall the PE stream
    """
    import concourse.mybir as mybir
    import concourse.tile as tile
    from concourse import bacc
    from concourse.masks import make_identity

    B = len(nt)
    G = 4  # query heads per core
    MAXNT = int(max(nt))
    off = np.concatenate([[0], np.cumsum(np.asarray(nt, dtype=np.int64))])
    # kv slab boundaries (in 128-token blocks): small slabs at the start so
    # compute begins ~1us in, ~2MB steady-state slabs for peak HBM bandwidth,
    # small slabs at the end to shorten the post-DMA drain.
    SLAB = slab if slab else (64 if fp8 else 32)
    sizes = [max(SLAB // 8, 1), max(SLAB // 4, 1), max(SLAB // 2, 1)]
    while sum(sizes) < nblk - SLAB - sum((SLAB // 4, SLAB // 8)):
        sizes.append(SLAB)
    sizes += [max(SLAB // 4, 1), max(SLAB // 8, 1)]
    bounds = [0]
    for s in sizes:
        if bounds[-1] >= nblk:
            break
        bounds.append(min(nblk, bounds[-1] + s))
    if bounds[-1] < nblk:
        bounds.append(nblk)
    NS = len(bounds) - 1  # number of kv slabs
    slab_of = np.searchsorted(np.asarray(bounds), np.arange(nblk), side="right") - 1
    f32 = mybir.dt.float32
    bf16 = mybir.dt.bfloat16
    kvdt = mybir.dt.float8e3 if fp8 else bf16

    nc = bacc.Bacc(None, target_bir_lowering=False)
    kv = nc.dram_tensor("kv", [P, nblk * 2 * P], kvdt, kind="ExternalInput")
    qh = nc.dram_tensor("qh", [P, B * G], bf16, kind="ExternalInput")  # [d,(b,g)]
    out = nc.dram_tensor("out", [B, G * P], f32, kind="ExternalOutput")

    with tile.TileContext(nc) as tc:
        with tc.tile_pool(name="persist", bufs=1) as persist:
            ident_f = persist.tile([P, P], f32)
            make_identity(nc, ident_f)
            ones_col_bf = persist.tile([P, 1], bf16)
            nc.vector.memset(ones_col_bf, 1.0)
            ones_row_f = persist.tile([1, P], f32)
            nc.vector.memset(ones_row_f, 1.0)
            # neg_tab[p, r] = 0.0 if p < r else -30.0 — fused into the exp as
            # a per-partition bias for the partial last tile (r = rem):
            # exp(scale*s - 30) ~ 1e-13 zeroes the padding rows without a
            # separate DVE mask op in the PV dependency chain.
            neg_tab = persist.tile([P, P + 1], f32)
            nc.gpsimd.memset(neg_tab, -30.0)
            nc.gpsimd.affine_select(
                out=neg_tab,
                in_=neg_tab,
                compare_op=mybir.AluOpType.is_ge,
                fill=0.0,
                base=0,
                pattern=[[-1, P + 1]],
                channel_multiplier=1,
            )
            qh_bf = persist.tile([P, B * G], bf16)
            nc.gpsimd.dma_start(qh_bf[:], qh[:])
            outT = persist.tile([P, B * G], f32)  # [d, (b,g)]
            l_red = persist.tile([1, B * G], f32)
            recip = persist.tile([1, B * G], f32)
            outN = persist.tile([P, B * G], f32)
            outF = persist.tile([P, B * G], f32)

            if no_dma or contend:
                # no_dma=True: one resident dummy slab. no_dma=2: rotate
                # through 8 dummy-slab regions so PE stationary loads hit
                # spread SBUF addresses like the real kernel's rotating pool.
                n_dummy = 8 if no_dma == 2 else 1
                dummy_big = persist.tile([P, n_dummy * SLAB * 2 * P], kvdt)
                W = SLAB * 2 * P
                for j in range(n_dummy):
                    nc.vector.memset(dummy_big[:, j * W : (j + 1) * W], 0.0)
                dummies = [
                    dummy_big[:, j * W : (j + 1) * W] for j in range(n_dummy)
                ]

            with (
                tc.tile_pool(name="kv", bufs=bufs) as kvpool,
                tc.tile_pool(
                    name="sc_ps", bufs=spbufs or pv_lag + 1, space="PSUM"
                ) as scps,
                tc.tile_pool(name="probs", bufs=pv_lag + 1) as prpool,
                tc.tile_pool(name="acc_ps", bufs=3, space="PSUM") as accps,
            ):
                state = {}
                slabs = {}

                dma_engines = [nc.gpsimd, nc.sync, nc.scalar]

                def get_slab(s):
                    """Rotating-pool slab load; emission order is monotone in
                    s because blocks are visited in concat order."""
                    if no_dma or contend:
                        return dummies[s % len(dummies)]
                    if s not in slabs:
                        st = kvpool.tile([P, SLAB * 2 * P], kvdt)
                        lo = bounds[s] * 2 * P
                        hi = bounds[s + 1] * 2 * P
                        if altq == -1:
                            eng = nc.sync
                        elif altq:
                            eng = dma_engines[s % altq]
                        else:
                            eng = nc.gpsimd
                        eng.dma_start(st[:, : hi - lo], kv[:, lo:hi])
                        slabs[s] = st
                    return slabs[s]

                def _kt_of(i):  # K^T [d, t] of global block i
                    s = int(slab_of[i])
                    r = i - bounds[s]
                    return get_slab(s)[:, r * 2 * P : r * 2 * P + P]

                def _vt_of(i):  # V [t, d] of global block i
                    s = int(slab_of[i])
                    r = i - bounds[s]
                    return get_slab(s)[:, r * 2 * P + P : (r + 1) * 2 * P]

                def emit_scores(b, pv_cb=None):
                    ntb = int(nt[b])
                    o = int(off[b])
                    scores = scps.tile([P, G * MAXNT], f32)
                    for _rep in range(2 if qk2x else 1):
                        for i in range(ntb):
                            if (
                                pv_cb is not None
                                and i > 0
                                and slab_of[o + i] != slab_of[o + i - 1]
                            ):
                                # the PE is in-order: place the previous
                                # sequence's (ready) PV work ahead of the
                                # chunks that will wait on the next slab
                                pv_cb()
                                pv_cb = None
                            nc.tensor.matmul(
                                scores[:, G * i : G * (i + 1)],
                                lhsT=_kt_of(o + i),
                                rhs=qh_bf[:, G * b : G * (b + 1)],
                                start=True,
                                stop=True,
                            )
                    pb = prpool.tile([P, G * MAXNT], bf16)
                    r = int(rem[b])
                    full = G * (ntb - 1) if r < P else G * ntb
                    if full:
                        nc.scalar.activation(
                            pb[:, :full],
                            scores[:, :full],
                            mybir.ActivationFunctionType.Exp,
                            scale=SCALE,
                        )
                    if r < P:
                        nc.scalar.activation(
                            pb[:, G * (ntb - 1) : G * ntb],
                            scores[:, G * (ntb - 1) : G * ntb],
                            mybir.ActivationFunctionType.Exp,
                            bias=neg_tab[:, r : r + 1],
                            scale=SCALE,
                        )
                    state[b] = pb

                def emit_pv(b):
                    ntb = int(nt[b])
                    o = int(off[b])
                    pb = state.pop(b)
                    lp = accps.tile([1, G * MAXNT], f32, tag="acc")
                    nc.tensor.matmul(
                        lp[:, : G * ntb],
                        lhsT=ones_col_bf,
                        rhs=pb[:, : G * ntb],
                        start=True,
                        stop=True,
                    )
                    otp = accps.tile([P, G], f32, tag="acc")
                    for i in range(ntb):
                        nc.tensor.matmul(
                            otp,
                            lhsT=_vt_of(o + i),
                            rhs=pb[:, G * i : G * (i + 1)],
                            start=(i == 0),
                            stop=(i == ntb - 1),
                        )
                    nc.vector.tensor_copy(outT[:, G * b : G * (b + 1)], otp)
                    nc.vector.tensor_reduce(
                        l_red[0:1, G * b : G * (b + 1)],
                        lp[0:1, : G * ntb].rearrange("p (i h) -> p h i", h=G),
                        axis=mybir.AxisListType.X,
                        op=mybir.AluOpType.add,
                    )

                def emit_loads_raw():
                    """Real slab loads with tiny consumers (no compute dep)."""
                    for s in range(NS):
                        st = kvpool.tile([P, SLAB * 2 * P], kvdt)
                        lo = bounds[s] * 2 * P
                        hi = bounds[s + 1] * 2 * P
                        if altq == -1:
                            eng = nc.sync
                        elif altq:
                            eng = dma_engines[s % altq]
                        else:
                            eng = nc.gpsimd
                        eng.dma_start(st[:, : hi - lo], kv[:, lo:hi])
                        nc.vector.tensor_copy(outT[0:1, s : s + 1], st[0:1, 0:1])

                def emit_body():
                    slabs.clear()
                    if contend:
                        # real DMA stream + full compute on a dummy slab, no
                        # cross-deps: isolates resource contention from
                        # dependency stalls
                        emit_loads_raw()
                        pend = []
                        for b in range(B):
                            emit_scores(b)
                            pend.append(b)
                            if len(pend) > pv_lag:
                                emit_pv(pend.pop(0))
                        for b in pend:
                            emit_pv(b)
                    elif dma_only:
                        emit_loads_raw()
                        nc.vector.memset(l_red, 1.0)
                    elif pipeline_pv:
                        pend = []
                        for b in range(B):
                            cb = None
                            if pvint and len(pend) >= pv_lag:
                                cb = lambda: emit_pv(pend.pop(0))
                            emit_scores(b, cb)
                            pend.append(b)
                            if len(pend) > pv_lag:
                                emit_pv(pend.pop(0))
                        for b in pend:
                            emit_pv(b)
                    else:
                        for b in range(B):
                            emit_scores(b)
                            emit_pv(b)

                if replay > 1:
                    with tc.For_i(0, replay, 1):
                        emit_body()
                else:
                    emit_body()

            # ---- finalize: out = outT / l, transposed to [(b,g), d] ----
            with tc.tile_pool(name="fin_ps", bufs=1, space="PSUM") as finps:
                nc.vector.reciprocal(recip, l_red)
                bc = finps.tile([P, B * G], f32)
                nc.tensor.matmul(
                    bc, lhsT=ones_row_f, rhs=recip, start=True, stop=True
                )
                nc.vector.tensor_mul(outN, outT, bc)
                tp2 = finps.tile([P, B * G], f32)
                nc.tensor.transpose(tp2, outN, ident_f)
                nc.vector.tensor_copy(outF, tp2)
                nc.sync.dma_start(
                    out.rearrange("b (g d) -> (b g) d", g=G), outF
                )
    nc.compile()
    return nc


def _prepare(
    query,
    key,
    value,
    key_cache,
    value_cache,
    block_tables,
    seq_lens,
    build=True,
    fp8=True,
):
    """Build the compiled SPMD graph and the per-core input shards."""
    import ml_dtypes

    bf16 = ml_dtypes.bfloat16
    kvdt = ml_dtypes.float8_e3m4 if fp8 else bf16
    query = np.asarray(query, dtype=np.float32)
    key = np.asarray(key, dtype=np.float32)
    value = np.asarray(value, dtype=np.float32)
    key_cache = np.asarray(key_cache, dtype=np.float32)
    value_cache = np.asarray(value_cache, dtype=np.float32)
    block_tables = np.asarray(block_tables)
    seq_lens = np.asarray(seq_lens)

    B, H, D = query.shape
    KVH = key.shape[1]
    NB, BS = key_cache.shape[0], key_cache.shape[1]
    S_MAX = block_tables.shape[1] * BS
    G = H // KVH
    N_CORES = 8
    assert KVH == N_CORES and D == P

    L = np.maximum(seq_lens.astype(np.int64), 1)
    # `order[s]` = original index of the sequence processed s-th; outputs
    # are unscrambled on the host.
    order = _seq_order(L)
    L = L[order]
    nt = ((L + P - 1) // P).astype(np.int64)  # tiles incl. the new token
    rem = L - (nt - 1) * P  # valid tokens in last tile (1..128)
    off = np.concatenate([[0], np.cumsum(nt)])
    TOT = int(off[-1]) * P

    kc_flat = key_cache.reshape(NB * BS, KVH, D)
    vc_flat = value_cache.reshape(NB * BS, KVH, D)

    # Token slot ids, concatenated per sequence (nt[b]*128 tokens each; the
    # tail past L is read-but-masked padding). With arange block tables (the
    # spec's fill) slot (b, t) is just b*S_MAX + t.
    arange_ok = bool(
        np.array_equal(
            block_tables.ravel(),
            np.arange(block_tables.size, dtype=block_tables.ravel().dtype),
        )
    )
    tok_idx = np.empty(TOT, np.int64)
    for b in range(B):
        ob = int(order[b])  # original sequence index
        t = np.arange(int(nt[b]) * P, dtype=np.int64)
        # tile padding past the sequence's allocated pages re-reads the last
        # valid slot (finite data; zeroed by the exp mask anyway)
        t = np.minimum(t, S_MAX - 1)
        if arange_ok:
            ids = ob * S_MAX + t
        else:
            ids = block_tables[ob, t // BS].astype(np.int64) * BS + t % BS
        tok_idx[off[b] * P : (off[b] + nt[b]) * P] = ids
    newpos = off[:-1] * P + (L - 1)  # new token position in the concat layout

    NBLK = int(off[-1])
    nc = _build_graph(nt, rem, NBLK, fp8=fp8) if build else None

    lim = float(ml_dtypes.finfo(kvdt).max)
    in_maps = []
    for c in range(N_CORES):
        k_sel = kc_flat[tok_idx, c, :]  # [TOT, D] f32
        v_sel = vc_flat[tok_idx, c, :]
        k_sel[newpos] = key[order, c, :]
        v_sel[newpos] = value[order, c, :]
        kt3 = k_sel.T.reshape(P, NBLK, P)  # [d, blk, t]
        vp3 = v_sel.reshape(NBLK, P, P).transpose(1, 0, 2)  # [p, blk, d]
        kv_c = np.ascontiguousarray(
            np.stack([kt3, vp3], axis=2)
            .reshape(P, NBLK * 2 * P)
            .clip(-lim, lim)
            .astype(kvdt)
        )
        qh_c = np.ascontiguousarray(
            query[order][:, c * G : (c + 1) * G, :]
            .transpose(2, 0, 1)
            .reshape(D, B * G)
            .astype(bf16)
        )
        in_maps.append({"kv": kv_c, "qh": qh_c})
    return nc, in_maps, (B, H, D, G), order


def kernel(query, key, value, key_cache, value_cache, block_tables, seq_lens):
    from concourse.bass_utils import run_bass_kernel_spmd

    nc, in_maps, (B, H, D, G), order = _prepare(
        query, key, value, key_cache, value_cache, block_tables, seq_lens
    )
    res = run_bass_kernel_spmd(nc, in_maps, core_ids=list(range(len(in_maps))))
    out = np.empty((B, H * D), np.float32)
    for c in range(len(in_maps)):
        out[order, c * G * D : (c + 1) * G * D] = res.results[c]["out"]
    return out
